# revision 1
# baseline (speedup 1.0000x reference)
"""GCN graph-classification kernel for 8 Trainium2 NeuronCores.

Model (PyG-style GCNConv x2 + mean pool + log_softmax):
    h   = x @ W1
    H1  = relu(Ahat @ h + b1)          Ahat = D^-1/2 (A + I) D^-1/2
    H2  = Ahat @ (H1 @ W2) + b2
    out = log_softmax(mean-pool-per-graph(H2))

Distribution strategy (8 cores):
  * nodes partitioned contiguously (6250/core); per-core in-degree-sorted
    permutation so destination tiles have homogeneous degrees.
  * layer 1: h computed locally, dis-prescaled, AllGathered; each core
    aggregates its own nodes' in-edges with dma_gather (edge messages) +
    one-hot selector matmuls accumulating in PSUM.
  * layer 2 + pooling folded:  pooled = (Q @ H1) @ W2 + b2  with
    Q = P_mean @ Ahat  (500 x 50000, built dense-per-node-tile on host).
    Each core contracts its own H1 tiles against its Q blocks -> partial
    per-graph sums -> AllReduce (500x128 floats) -> W2 -> log_softmax.
  All symmetric-norm factors, mean-pool counts and the permutation are
  folded into host-built index/selector/Q arrays (pure index-side prep).
"""

import os
import numpy as np

import concourse.bacc as bacc
import concourse.mybir as mybir
from concourse import tile
from concourse.bass_utils import run_bass_kernel_spmd

# ---------------------------------------------------------------- constants
N, E, F, HID, C, G = 50000, 600000, 128, 128, 16, 500
P = 8                      # NeuronCores
NV = N // P                # nodes per core
NT = (NV + 127) // 128     # node tiles per core (49)
TPAD = NT * 128            # padded per-core node count (6272)
GP = 512                   # padded graph count
GT = GP // 128             # graph tiles
HALF = N // 2              # gather-table half size (int16 index limit)
NB = 7                     # layer-1 gather batches (NT % NB == 0)

AF = mybir.ActivationFunctionType
ALU = mybir.AluOpType

LAST_EXEC_NS = None
LAST_RESULT = None


def _install_profile_hook():
    """The agent image's antenv lacks axon_hooks; shim it so
    run_bass_kernel_spmd(trace=True) can capture NTFF profiles."""
    import sys
    import types
    if "antenv.axon_hooks" in sys.modules:
        return True
    try:
        from trn_agent_boot.trn_boot import _ntff_profile_via_ctypes
        hook = _ntff_profile_via_ctypes("/opt/axon/libaxon_pjrt.so")
        if hook is None:
            return False
        mod = types.ModuleType("antenv.axon_hooks")
        mod._hook = hook
        mod.get_axon_ntff_profile_hook = lambda: mod._hook

        def _set(h):
            mod._hook = h
        mod.set_axon_ntff_profile_hook = _set
        sys.modules["antenv.axon_hooks"] = mod
        import antenv
        antenv.axon_hooks = mod
        return True
    except Exception as e:  # profiling is best-effort
        print(f"profile hook unavailable: {e}")
        return False


# ---------------------------------------------------------------- host prep
def _preprocess(x, W1, b1, W2, b2, edge_src, edge_dst, batch):
    f32 = np.float32
    src = np.asarray(edge_src, np.int64)
    dst = np.asarray(edge_dst, np.int64)
    bat = np.asarray(batch, np.int64)
    x = np.asarray(x, f32)

    deg = np.bincount(dst, minlength=N).astype(np.float64) + 1.0
    dis = 1.0 / np.sqrt(deg)
    cnt = np.maximum(np.bincount(bat, minlength=G), 1).astype(np.float64)

    # per-core degree-descending node permutation
    pos = np.empty(N, np.int64)
    order = np.empty(N, np.int64)      # order[k*NV+j] = node at position j
    for k in range(P):
        v0 = k * NV
        loc = np.argsort(-deg[v0:v0 + NV], kind="stable")
        order[v0:v0 + NV] = v0 + loc
        pos[v0 + loc] = np.arange(NV)
    slot = (np.arange(N) // NV) * NV + pos     # row of node in AllGathered h

    # ---- layer-1 edges (incl. self-loops), grouped (core, tile, src-half)
    e_src = np.concatenate([src, np.arange(N)])
    e_dst = np.concatenate([dst, np.arange(N)])
    d_own = e_dst // NV
    d_pos = pos[e_dst]
    t_of = d_pos // 128
    dloc_v = (d_pos % 128).astype(f32)
    sslot = slot[e_src]
    is_hi = (sslot >= HALF).astype(np.int64)
    idx_v = (sslot - is_hi * HALF).astype(np.int16)

    key = (d_own * NT + t_of) * 2 + is_hi
    ordr = np.argsort(key, kind="stable")
    idx_s = idx_v[ordr]
    dloc_s = dloc_v[ordr]
    bounds = np.searchsorted(key[ordr], np.arange(P * NT * 2 + 1))
    cnts = np.diff(bounds).reshape(P, NT, 2)
    CH = -(-cnts // 128)               # chunks per (core, tile, half)
    CH = CH.max(axis=0)                # [NT, 2]  uniform across cores

    # batches: stride-interleaved tiles so per-batch work is balanced
    tiles_of_batch = [[t for t in range(NT) if t % NB == b] for b in range(NB)]

    # chunk-column / gather-index layout (shared by all cores)
    # per batch: [lo chunks tile-major][hi chunks tile-major]
    chunk_specs = []       # (tile, half, batch, col, gslice_pos) per chunk
    batch_meta = []        # per batch: dict(nlo, nhi, col0, icol_lo, icol_hi)
    col = 0
    icol = 0
    for b in range(NB):
        nlo = int(sum(CH[t, 0] for t in tiles_of_batch[b]))
        nhi = int(sum(CH[t, 1] for t in tiles_of_batch[b]))
        meta = dict(nlo=nlo, nhi=nhi, col0=col,
                    icol_lo=icol, icol_hi=icol + nlo * 8)
        batch_meta.append(meta)
        j = 0
        for h in (0, 1):
            for t in tiles_of_batch[b]:
                for _ in range(int(CH[t, h])):
                    chunk_specs.append((t, h, b, col, j))
                    col += 1
                    j += 1
        icol += (nlo + nhi) * 8
    NCH = col
    NIDX = NCH * 128

    # per-core data arrays
    xT = np.zeros((P, 128, TPAD), f32)
    disc = np.zeros((P, 128, NT), f32)
    qb = np.zeros((P, TPAD, GP), f32)
    dloc_all = np.full((P, 128, NCH), -1.0, f32)
    idx_flat = np.zeros((P, NIDX), np.int16)

    for k in range(P):
        ok = order[k * NV:(k + 1) * NV]
        xT[k, :, :NV] = x[ok].T
        d = np.zeros(TPAD, f32)
        d[:NV] = dis[ok].astype(f32)
        disc[k] = d.reshape(NT, 128).T

    # fill chunk idx / dloc tables
    for b in range(NB):
        m = batch_meta[b]
        for h, base_icol, nch_h in ((0, m["icol_lo"], m["nlo"]),
                                    (1, m["icol_hi"], m["nhi"])):
            jh = 0
            for t in tiles_of_batch[b]:
                nchunk = int(CH[t, h])
                if nchunk > 0:
                    for k in range(P):
                        gi = (k * NT + t) * 2 + h
                        g0, g1 = bounds[gi], bounds[gi + 1]
                        n = g1 - g0
                        fbase = base_icol * 16 + jh * 128
                        idx_flat[k, fbase:fbase + n] = idx_s[g0:g1]
                        pp = np.arange(n) % 128
                        cc = np.arange(n) // 128
                        # chunk columns for this (t,h) block
                        colbase = m["col0"] + (0 if h == 0 else m["nlo"]) + jh
                        dloc_all[k, pp, colbase + cc] = dloc_s[g0:g1]
                jh += nchunk
    assert idx_flat.min() >= 0 and int(idx_flat.max()) < HALF
    # wrap gather indices: i -> [i % 16, i // 16], replicated to 128 partitions
    idxs = np.tile(
        idx_flat.reshape(P, NIDX // 16, 16).transpose(0, 2, 1), (1, 8, 1)
    ).astype(np.int16)

    # ---- layer-2 Q blocks: qb[core, pos[src], g] += norm/cnt[g]
    g_of = bat[e_dst]
    val = (dis[e_src] * dis[e_dst] / cnt[g_of]).astype(f32)
    np.add.at(qb, (e_src // NV, pos[e_src], g_of), val)

    iota2d = np.broadcast_to(
        np.arange(128, dtype=f32), (128, 128)).copy()
    eye16 = np.eye(16, dtype=f32)

    import ml_dtypes
    qb = qb.astype(ml_dtypes.bfloat16)

    W1 = np.ascontiguousarray(np.asarray(W1, f32))
    W2 = np.ascontiguousarray(np.asarray(W2, f32))
    b1 = np.asarray(b1, f32)
    b2 = np.asarray(b2, f32)
    use_b1 = bool(np.any(b1))
    use_b2 = bool(np.any(b2))

    in_maps = []
    for k in range(P):
        m = {
            "xT": np.ascontiguousarray(xT[k]),
            "qb": np.ascontiguousarray(qb[k]),
            "idxs": np.ascontiguousarray(idxs[k]),
            "dloc": np.ascontiguousarray(dloc_all[k]),
            "disc": np.ascontiguousarray(disc[k]),
            "w1": W1, "w2": W2,
            "iota": iota2d, "eye16": eye16,
        }
        if use_b1:
            rr = np.zeros((1, TPAD), f32)
            rr[0, :NV] = np.sqrt(deg[order[k * NV:(k + 1) * NV]]).astype(f32)
            m["rdis"] = rr
            m["b1r"] = b1.reshape(1, F)
        if use_b2:
            m["b2r"] = b2.reshape(1, C)
        in_maps.append(m)

    plan = dict(NCH=NCH, NIDX=NIDX, CH=CH, tiles_of_batch=tiles_of_batch,
                chunk_specs=chunk_specs, batch_meta=batch_meta,
                use_b1=use_b1, use_b2=use_b2)
    return plan, in_maps


# ---------------------------------------------------------------- bass build
def _build(plan):
    dt = mybir.dt
    f32, bf16, i16 = dt.float32, dt.bfloat16, dt.int16
    NCH, NIDX = plan["NCH"], plan["NIDX"]
    use_b1, use_b2 = plan["use_b1"], plan["use_b2"]
    CH = plan["CH"]

    stage = int(os.environ.get("GCN_STAGE", "3"))  # 1: no phase C; 2: +gathers
    nc = bacc.Bacc("TRN2", target_bir_lowering=False, debug=False,
                   num_devices=P)
    xT_d = nc.dram_tensor("xT", [128, TPAD], f32, kind="ExternalInput")
    qb_d = nc.dram_tensor("qb", [TPAD, GP], bf16, kind="ExternalInput")
    idxs_d = nc.dram_tensor("idxs", [128, NIDX // 16], i16, kind="ExternalInput")
    dloc_d = nc.dram_tensor("dloc", [128, NCH], f32, kind="ExternalInput")
    disc_d = nc.dram_tensor("disc", [128, NT], f32, kind="ExternalInput")
    w1_d = nc.dram_tensor("w1", [F, HID], f32, kind="ExternalInput")
    w2_d = nc.dram_tensor("w2", [HID, C], f32, kind="ExternalInput")
    iota_d = nc.dram_tensor("iota", [128, 128], f32, kind="ExternalInput")
    eye_d = nc.dram_tensor("eye16", [16, 16], f32, kind="ExternalInput")
    if use_b1:
        rdis_d = nc.dram_tensor("rdis", [1, TPAD], f32, kind="ExternalInput")
        b1_d = nc.dram_tensor("b1r", [1, F], f32, kind="ExternalInput")
    if use_b2:
        b2_d = nc.dram_tensor("b2r", [1, C], f32, kind="ExternalInput")
    y_d = nc.dram_tensor("y", [G, C], f32, kind="ExternalOutput")

    with tile.TileContext(nc) as tc:
        cpool = tc.alloc_tile_pool(name="const", bufs=1)
        dram = tc.alloc_tile_pool(name="dram", bufs=1, space="DRAM")

        w1_sb = cpool.tile([F, HID], f32)
        nc.sync.dma_start(w1_sb[:], w1_d[:, :])
        w2_sb = cpool.tile([HID, C], f32)
        nc.sync.dma_start(w2_sb[:], w2_d[:, :])
        disc_sb = cpool.tile([128, NT], f32)
        nc.sync.dma_start(disc_sb[:], disc_d[:, :])
        iota_sb = cpool.tile([128, 128], f32)
        nc.sync.dma_start(iota_sb[:], iota_d[:, :])
        eye_sb = cpool.tile([16, 16], f32)
        nc.sync.dma_start(eye_sb[:], eye_d[:, :])
        idxs_sb = cpool.tile([128, NIDX // 16], i16)
        nc.sync.dma_start(idxs_sb[:], idxs_d[:, :])
        dloc_sb = cpool.tile([128, NCH], f32)
        nc.sync.dma_start(dloc_sb[:], dloc_d[:, :])
        h1_sb = cpool.tile([128, TPAD], bf16)
        if use_b1:
            rdis_sb = cpool.tile([1, TPAD], f32)
            nc.sync.dma_start(rdis_sb[:], rdis_d[:, :])
            b1_sb = cpool.tile([1, F], f32)
            nc.sync.dma_start(b1_sb[:], b1_d[:, :])
        if use_b2:
            b2_sb = cpool.tile([1, C], f32)
            nc.sync.dma_start(b2_sb[:], b2_d[:, :])

        h_own = dram.tile([NV, F], f32)
        h_full = dram.tile([N, F], f32)
        ar_in = dram.tile([128, GP], f32)
        ar_out = dram.tile([128, GP], f32)

        # ---------------- phase B: h = dis * (x @ W1), AllGather
        with (
            tc.tile_pool(name="xw", bufs=1) as xw,
            tc.tile_pool(name="hp", bufs=2, space="PSUM") as hp,
            tc.tile_pool(name="ht", bufs=3) as htp,
        ):
            xT_sb = xw.tile([128, TPAD], f32)
            nc.sync.dma_start(xT_sb[:], xT_d[:, :])
            for t in range(NT):
                ps = hp.tile([128, 128], f32)
                nc.tensor.matmul(ps[:], lhsT=xT_sb[:, t * 128:(t + 1) * 128],
                                 rhs=w1_sb[:], start=True, stop=True)
                ht = htp.tile([128, 128], f32)
                nc.scalar.activation(ht[:], ps[:], AF.Copy,
                                     scale=disc_sb[:, t:t + 1])
                rows = min(128, NV - t * 128)
                nc.sync.dma_start(h_own[t * 128:t * 128 + rows, :],
                                  ht[0:rows, :])

        nc.gpsimd.collective_compute(
            "AllGather", ALU.bypass, replica_groups=[list(range(P))],
            ins=[h_own[:].opt()], outs=[h_full[:].opt()])

        # ---------------- phase C: layer-1 aggregation + layer-2 contraction
        with tc.tile_pool(name="ptp", bufs=1, space="PSUM") as ptp:
            poolT = ptp.tile([128, GP], f32)
            i_l2 = 0
            with (
                tc.tile_pool(name="glo", bufs=2) as glo_p,
                tc.tile_pool(name="ghi", bufs=2) as ghi_p,
                tc.tile_pool(name="selp", bufs=8) as selp,
                tc.tile_pool(name="qp", bufs=3) as qp,
                tc.tile_pool(name="aggp", bufs=7, space="PSUM") as aggp,
            ):
                for b in range(NB):
                    m = plan["batch_meta"][b]
                    nlo, nhi = m["nlo"], m["nhi"]
                    ngb = int(os.environ.get("GCN_NGB", str(NB)))
                    glo = ghi = None
                    if b >= ngb:
                        continue
                    if nlo and stage >= 2:
                        glo = glo_p.tile([128, nlo, 128], f32, tag="glo")
                        nc.gpsimd.dma_gather(
                            out_ap=glo[:], in_ap=h_full[0:HALF, :],
                            idxs_ap=idxs_sb[:, m["icol_lo"]:
                                            m["icol_lo"] + nlo * 8],
                            num_idxs=nlo * 128, num_idxs_reg=nlo * 128,
                            elem_size=F, single_packet=False)
                    if nhi and stage >= 2:
                        ghi = ghi_p.tile([128, nhi, 128], f32, tag="ghi")
                        nc.gpsimd.dma_gather(
                            out_ap=ghi[:], in_ap=h_full[HALF:N, :],
                            idxs_ap=idxs_sb[:, m["icol_hi"]:
                                            m["icol_hi"] + nhi * 8],
                            num_idxs=nhi * 128, num_idxs_reg=nhi * 128,
                            elem_size=F, single_packet=False)
                    if stage < 3:
                        if stage == 2 and (glo is not None or ghi is not None):
                            junk = selp.tile([128, 128], f32, tag="sel")
                            gj = glo if glo is not None else ghi
                            nc.vector.tensor_copy(junk[:], gj[:, 0, :])
                            nc.sync.dma_start(ar_in[0:128, 0:128], junk[:])
                        continue
                    # chunks of this batch, grouped per tile
                    per_tile = {}
                    for (t, h, bb, ccol, j) in plan["chunk_specs"]:
                        if bb == b:
                            per_tile.setdefault(t, []).append((h, ccol, j))
                    for t in plan["tiles_of_batch"][b]:
                        chunks = per_tile[t]
                        ps = aggp.tile([128, 128], f32, tag="agg")
                        first = True
                        if use_b1:
                            nc.tensor.matmul(
                                ps[:], lhsT=rdis_sb[0:1, t * 128:(t + 1) * 128],
                                rhs=b1_sb[:], start=True, stop=False)
                            first = False
                        for ci, (h, ccol, j) in enumerate(chunks):
                            sel = selp.tile([128, 128], f32, tag="sel")
                            nc.vector.tensor_tensor(
                                out=sel[:], in0=iota_sb[:],
                                in1=dloc_sb[:, ccol:ccol + 1].to_broadcast(
                                    [128, 128]),
                                op=ALU.is_equal)
                            gsrc = ghi if h else glo
                            joff = (j - nlo) if h else j
                            nc.tensor.matmul(
                                ps[:], lhsT=sel[:], rhs=gsrc[:, joff, :],
                                start=first, stop=(ci == len(chunks) - 1))
                            first = False
                        nc.scalar.activation(
                            h1_sb[:, t * 128:(t + 1) * 128], ps[:], AF.Relu,
                            scale=disc_sb[:, t:t + 1])
                        # layer 2: poolT += H1_tile^T-contraction with Q block
                        qt = qp.tile([128, GP], bf16, tag="q")
                        nc.sync.dma_start(
                            qt[:], qb_d[t * 128:(t + 1) * 128, :])
                        nc.tensor.matmul(
                            poolT[:],
                            lhsT=h1_sb[:, t * 128:(t + 1) * 128],
                            rhs=qt[:],
                            start=(i_l2 == 0), stop=(i_l2 == NT - 1))
                        i_l2 += 1

            pt_sb = cpool.tile([128, GP], f32)
            if stage >= 3:
                nc.scalar.activation(pt_sb[:], poolT[:], AF.Copy)
            else:
                nc.vector.memset(pt_sb[:], 0.0)
            nc.sync.dma_start(ar_in[:], pt_sb[:])

        nc.gpsimd.collective_compute(
            "AllReduce", ALU.add, replica_groups=[list(range(P))],
            ins=[ar_in[:].opt()], outs=[ar_out[:].opt()])

        # ---------------- phase D: W2, bias, log_softmax
        with (
            tc.tile_pool(name="fin", bufs=1) as fin,
            tc.tile_pool(name="fps", bufs=2, space="PSUM") as fps,
            tc.tile_pool(name="sm", bufs=4) as smp,
        ):
            pooledT = fin.tile([128, GP], f32)
            nc.sync.dma_start(pooledT[:], ar_out[:])
            out2 = fps.tile([16, GP], f32, tag="out2")
            nc.tensor.matmul(out2[:], lhsT=w2_sb[:], rhs=pooledT[:],
                             start=True, stop=not use_b2)
            if use_b2:
                ones = fin.tile([1, GP], f32)
                nc.vector.memset(ones[:], 1.0)
                nc.tensor.matmul(out2[:], lhsT=b2_sb[:], rhs=ones[:],
                                 start=False, stop=True)
            logitsT = fin.tile([16, GP], f32)
            nc.scalar.activation(logitsT[:], out2[:], AF.Copy)
            for gt in range(min(GT, -(-G // 128))):
                tp = fps.tile([128, 16], f32, tag="tp")
                nc.tensor.transpose(
                    tp[:], logitsT[:, gt * 128:(gt + 1) * 128], eye_sb[:])
                nmx = smp.tile([128, 1], f32, tag="nmx")
                nc.vector.reduce_max(out=nmx[:], in_=tp[:],
                                     axis=mybir.AxisListType.X, negate=True)
                ex = smp.tile([128, 16], f32, tag="ex")
                nc.scalar.activation(ex[:], tp[:], AF.Exp, bias=nmx[:, 0:1])
                sm = smp.tile([128, 1], f32, tag="sm")
                nc.vector.reduce_sum(out=sm[:], in_=ex[:],
                                     axis=mybir.AxisListType.X)
                lse = smp.tile([128, 1], f32, tag="lse")
                nc.scalar.activation(lse[:], sm[:], AF.Ln)
                res = smp.tile([128, 16], f32, tag="res")
                nc.vector.tensor_scalar(res[:], tp[:], nmx[:, 0:1],
                                        lse[:, 0:1], ALU.add, ALU.subtract)
                rows = min(128, G - gt * 128)
                nc.sync.dma_start(y_d[gt * 128:gt * 128 + rows, :],
                                  res[0:rows, :])
        dram.release()
        cpool.release()
    nc.compile()
    return nc


# ---------------------------------------------------------------- entry
def kernel(x, W1, b1, W2, b2, edge_src, edge_dst, batch):
    global LAST_EXEC_NS, LAST_RESULT
    plan, in_maps = _preprocess(x, W1, b1, W2, b2,
                                edge_src, edge_dst, batch)
    nc = _build(plan)
    trace = bool(int(os.environ.get("GCN_TRACE", "0")))
    kw = {}
    if trace and _install_profile_hook():
        kw = dict(trace=True, trace_cores=[0])
    res = run_bass_kernel_spmd(nc, in_maps, core_ids=list(range(P)), **kw)
    LAST_RESULT = res
    LAST_EXEC_NS = res.exec_time_ns
    return np.ascontiguousarray(res.results[0]["y"].astype(np.float32))



# revision 4
# speedup vs baseline: 2.0277x; 2.0277x over previous
"""GCN graph-classification kernel for 8 Trainium2 NeuronCores.

Model (PyG-style GCNConv x2 + mean pool + log_softmax):
    h   = x @ W1
    H1  = relu(Ahat @ h + b1)          Ahat = D^-1/2 (A + I) D^-1/2
    H2  = Ahat @ (H1 @ W2) + b2
    out = log_softmax(mean-pool-per-graph(H2))

Distribution strategy (8 cores):
  * nodes partitioned contiguously (6250/core); per-core in-degree-sorted
    permutation so destination tiles have homogeneous degrees.
  * layer 1: h computed locally (bf16), dis-prescaled, AllGathered
    (Shared-output HBM collective); each core aggregates its own nodes'
    in-edges with dma_gather (256B bf16 edge messages) + one-hot selector
    matmuls accumulating in PSUM. Gathers are spread round-robin across
    4 SWDGE queues so descriptor generation runs on 4 Q7 core-pairs in
    parallel. Self-loops are folded into one diag-matmul per tile from
    the locally kept h tiles (no gather traffic).
  * layer 2 + pooling folded:  pooled = (Q @ H1) @ W2 + b2  with
    Q = P_mean @ Ahat  (500 x 50000, built dense-per-node-tile on host).
    Each core contracts its own H1 tiles against its Q blocks -> partial
    per-graph sums -> AllReduce (500x128 floats) -> W2 -> log_softmax.
  All symmetric-norm factors, mean-pool counts and the permutation are
  folded into host-built index/selector/Q arrays (pure index-side prep).
"""

import os
import numpy as np

import concourse.bacc as bacc
import concourse.mybir as mybir
from concourse import tile
from concourse.bass_utils import run_bass_kernel_spmd

# ---------------------------------------------------------------- constants
N, E, F, HID, C, G = 50000, 600000, 128, 128, 16, 500
P = 8                      # NeuronCores
NV = N // P                # nodes per core
NT = (NV + 127) // 128     # node tiles per core (49)
TPAD = NT * 128            # padded per-core node count (6272)
GP = 512                   # padded graph count
GT = GP // 128             # graph tiles
HALF = N // 2              # gather-table half size (int16 index limit)
NB = 7                     # layer-1 gather batches
NQ = 4                     # SWDGE queues (parallel gather descriptor gen)

AF = mybir.ActivationFunctionType
ALU = mybir.AluOpType

LAST_EXEC_NS = None
LAST_RESULT = None


def _install_profile_hook():
    """The agent image's antenv lacks axon_hooks; shim it so
    run_bass_kernel_spmd(trace=True) can capture NTFF profiles."""
    import sys
    import types
    if "antenv.axon_hooks" in sys.modules:
        return True
    try:
        from trn_agent_boot.trn_boot import _ntff_profile_via_ctypes
        hook = _ntff_profile_via_ctypes("/opt/axon/libaxon_pjrt.so")
        if hook is None:
            return False
        mod = types.ModuleType("antenv.axon_hooks")
        mod._hook = hook
        mod.get_axon_ntff_profile_hook = lambda: mod._hook

        def _set(h):
            mod._hook = h
        mod.set_axon_ntff_profile_hook = _set
        sys.modules["antenv.axon_hooks"] = mod
        import antenv
        antenv.axon_hooks = mod
        return True
    except Exception as e:  # profiling is best-effort
        print(f"profile hook unavailable: {e}")
        return False


# ---------------------------------------------------------------- host prep
def _preprocess(x, W1, b1, W2, b2, edge_src, edge_dst, batch):
    import ml_dtypes
    f32 = np.float32
    bf16 = ml_dtypes.bfloat16
    src = np.asarray(edge_src, np.int64)
    dst = np.asarray(edge_dst, np.int64)
    bat = np.asarray(batch, np.int64)
    x = np.asarray(x, f32)

    deg = np.bincount(dst, minlength=N).astype(np.float64) + 1.0
    dis = 1.0 / np.sqrt(deg)
    cnt = np.maximum(np.bincount(bat, minlength=G), 1).astype(np.float64)

    # per-core degree-descending node permutation
    pos = np.empty(N, np.int64)
    order = np.empty(N, np.int64)      # order[k*NV+j] = node at position j
    for k in range(P):
        v0 = k * NV
        loc = np.argsort(-deg[v0:v0 + NV], kind="stable")
        order[v0:v0 + NV] = v0 + loc
        pos[v0 + loc] = np.arange(NV)
    slot = (np.arange(N) // NV) * NV + pos     # row of node in AllGathered h

    # ---- layer-1 gather edges (no self-loops; those come from local h
    # tiles via a diag matmul), grouped (core, tile, src-half)
    d_own = dst // NV
    d_pos = pos[dst]
    t_of = d_pos // 128
    dloc_v = (d_pos % 128).astype(f32)
    sslot = slot[src]
    is_hi = (sslot >= HALF).astype(np.int64)
    idx_v = (sslot - is_hi * HALF).astype(np.int16)

    key = (d_own * NT + t_of) * 2 + is_hi
    ordr = np.argsort(key, kind="stable")
    idx_s = idx_v[ordr]
    dloc_s = dloc_v[ordr]
    bounds = np.searchsorted(key[ordr], np.arange(P * NT * 2 + 1))
    cnts = np.diff(bounds).reshape(P, NT, 2)
    CH = -(-cnts // 128)               # chunks per (core, tile, half)
    CH = CH.max(axis=0)                # [NT, 2]  uniform across cores

    # batches: stride-interleaved tiles so per-batch work is balanced
    tiles_of_batch = [[t for t in range(NT) if t % NB == b] for b in range(NB)]

    # chunk-column layout: per batch, chunks grouped PER TILE (lo then hi)
    # so each tile's selector build is one contiguous is_eq op.
    # gather-index layout: per batch, [lo chunks tile-major][hi chunks
    # tile-major] (matches the two dma_gather calls).
    chunk_specs = []       # (tile, half, batch, col, gslice_pos) per chunk
    batch_meta = []        # per batch: dict(nlo, nhi, col0, icol_lo, icol_hi)
    tile_meta = {}         # per tile: dict(col0, nlo, nhi)
    col = 0
    icol = 0
    for b in range(NB):
        nlo = int(sum(CH[t, 0] for t in tiles_of_batch[b]))
        nhi = int(sum(CH[t, 1] for t in tiles_of_batch[b]))
        meta = dict(nlo=nlo, nhi=nhi,
                    icol_lo=icol, icol_hi=icol + nlo * 8)
        batch_meta.append(meta)
        # gather slice positions (j) are tile-major within each half
        jlo = 0
        jhi = 0
        for t in tiles_of_batch[b]:
            tile_meta[t] = dict(col0=col, nlo=int(CH[t, 0]), nhi=int(CH[t, 1]),
                                jlo=jlo, jhi=jhi)
            for _ in range(int(CH[t, 0])):
                chunk_specs.append((t, 0, b, col, jlo))
                col += 1
                jlo += 1
            for _ in range(int(CH[t, 1])):
                chunk_specs.append((t, 1, b, col, jhi))
                col += 1
                jhi += 1
        icol += (nlo + nhi) * 8
    NCH = col
    NIDX = NCH * 128

    # per-core data arrays
    xT = np.zeros((P, 128, TPAD), bf16)
    disc = np.zeros((P, 128, NT), f32)
    qb = np.zeros((P, TPAD, GP), f32)
    dloc_all = np.full((P, 128, NCH), -1.0, bf16)
    idx_flat = np.zeros((P, NIDX), np.int16)

    rng = np.arange(128)
    for k in range(P):
        ok = order[k * NV:(k + 1) * NV]
        xT[k, :, :NV] = x[ok].T.astype(bf16)
        d = np.zeros(TPAD, f32)
        d[:NV] = dis[ok].astype(f32)
        disc[k] = d.reshape(NT, 128).T

    # fill chunk idx / dloc tables (idx layout: per batch, lo tile-major
    # then hi tile-major; dloc layout: per batch, per tile lo then hi)
    for b in range(NB):
        m = batch_meta[b]
        for h, base_icol in ((0, m["icol_lo"]), (1, m["icol_hi"])):
            jh = 0
            for t in tiles_of_batch[b]:
                nchunk = int(CH[t, h])
                if nchunk > 0:
                    tm = tile_meta[t]
                    for k in range(P):
                        gi = (k * NT + t) * 2 + h
                        g0, g1 = bounds[gi], bounds[gi + 1]
                        n = g1 - g0
                        fbase = base_icol * 16 + jh * 128
                        idx_flat[k, fbase:fbase + n] = idx_s[g0:g1]
                        pp = np.arange(n) % 128
                        cc = np.arange(n) // 128
                        colbase = tm["col0"] + (0 if h == 0 else tm["nlo"])
                        dloc_all[k, pp, colbase + cc] = \
                            dloc_s[g0:g1].astype(bf16)
                jh += nchunk
    assert idx_flat.min() >= 0 and int(idx_flat.max()) < HALF
    # wrap gather indices: i -> [i % 16, i // 16], replicated to 128 partitions
    idxs = np.tile(
        idx_flat.reshape(P, NIDX // 16, 16).transpose(0, 2, 1), (1, 8, 1)
    ).astype(np.int16)

    # ---- layer-2 Q blocks: qb[core, pos[src], g] += norm/cnt[g]
    # (self-loops included here)
    e_src = np.concatenate([src, np.arange(N)])
    e_dst = np.concatenate([dst, np.arange(N)])
    g_of = bat[e_dst]
    val = (dis[e_src] * dis[e_dst] / cnt[g_of]).astype(f32)
    np.add.at(qb, (e_src // NV, pos[e_src], g_of), val)
    qb = qb.astype(bf16)

    iota_bf = np.broadcast_to(
        np.arange(128, dtype=bf16), (128, 128)).copy()
    eye16 = np.eye(16, dtype=f32)
    eye128 = np.eye(128, dtype=bf16)

    W1 = np.ascontiguousarray(np.asarray(W1, f32).astype(bf16))
    W2 = np.ascontiguousarray(np.asarray(W2, f32))
    b1 = np.asarray(b1, f32)
    b2 = np.asarray(b2, f32)
    use_b1 = bool(np.any(b1))
    use_b2 = bool(np.any(b2))

    in_maps = []
    for k in range(P):
        m = {
            "xT": np.ascontiguousarray(xT[k]),
            "qb": np.ascontiguousarray(qb[k]),
            "idxs": np.ascontiguousarray(idxs[k]),
            "dloc": np.ascontiguousarray(dloc_all[k]),
            "disc": np.ascontiguousarray(disc[k]),
            "eye128": eye128,
            "w1": W1, "w2": W2,
            "iota": iota_bf, "eye16": eye16,
        }
        if use_b1:
            rr = np.zeros((1, TPAD), f32)
            rr[0, :NV] = np.sqrt(deg[order[k * NV:(k + 1) * NV]]).astype(f32)
            m["rdis"] = rr
            m["b1r"] = b1.reshape(1, F)
        if use_b2:
            m["b2r"] = b2.reshape(1, C)
        in_maps.append(m)

    plan = dict(NCH=NCH, NIDX=NIDX, CH=CH, tiles_of_batch=tiles_of_batch,
                chunk_specs=chunk_specs, batch_meta=batch_meta,
                tile_meta=tile_meta, use_b1=use_b1, use_b2=use_b2)
    return plan, in_maps


# ---------------------------------------------------------------- bass build
def _build(plan):
    dt = mybir.dt
    f32, bf16, i16 = dt.float32, dt.bfloat16, dt.int16
    NCH, NIDX = plan["NCH"], plan["NIDX"]
    use_b1, use_b2 = plan["use_b1"], plan["use_b2"]

    stage = int(os.environ.get("GCN_STAGE", "3"))  # 1: no phase C; 2: +gathers
    nc = bacc.Bacc("TRN2", target_bir_lowering=False, debug=False,
                   num_devices=P, num_swdge_queues=NQ)
    xT_d = nc.dram_tensor("xT", [128, TPAD], bf16, kind="ExternalInput")
    qb_d = nc.dram_tensor("qb", [TPAD, GP], bf16, kind="ExternalInput")
    idxs_d = nc.dram_tensor("idxs", [128, NIDX // 16], i16, kind="ExternalInput")
    dloc_d = nc.dram_tensor("dloc", [128, NCH], bf16, kind="ExternalInput")
    disc_d = nc.dram_tensor("disc", [128, NT], f32, kind="ExternalInput")
    eye128_d = nc.dram_tensor("eye128", [128, 128], bf16, kind="ExternalInput")
    w1_d = nc.dram_tensor("w1", [F, HID], bf16, kind="ExternalInput")
    w2_d = nc.dram_tensor("w2", [HID, C], f32, kind="ExternalInput")
    iota_d = nc.dram_tensor("iota", [128, 128], bf16, kind="ExternalInput")
    eye_d = nc.dram_tensor("eye16", [16, 16], f32, kind="ExternalInput")
    if use_b1:
        rdis_d = nc.dram_tensor("rdis", [1, TPAD], f32, kind="ExternalInput")
        b1_d = nc.dram_tensor("b1r", [1, F], f32, kind="ExternalInput")
    if use_b2:
        b2_d = nc.dram_tensor("b2r", [1, C], f32, kind="ExternalInput")
    y_d = nc.dram_tensor("y", [G, C], f32, kind="ExternalOutput")

    with tile.TileContext(nc) as tc:
        cpool = tc.alloc_tile_pool(name="const", bufs=1)
        dram = tc.alloc_tile_pool(name="dram", bufs=1, space="DRAM")

        h_own = dram.tile([NV, F], bf16)
        h_full = dram.tile([N, F], bf16, addr_space="Shared")
        ar_in = dram.tile([128, GP], f32)
        ar_out = dram.tile([128, GP], f32)

        # phase-B-critical constants first so their DMAs run first
        w1_sb = cpool.tile([F, HID], bf16)
        nc.sync.dma_start(w1_sb[:], w1_d[:, :])
        disc_sb = cpool.tile([128, NT], f32)
        nc.sync.dma_start(disc_sb[:], disc_d[:, :])
        h_loc = cpool.tile([128, TPAD], bf16)   # local dis*h, node-major tiles
        h1_sb = cpool.tile([128, TPAD], bf16)

        # ---------------- phase B: h = dis * (x @ W1), AllGather
        with (
            tc.tile_pool(name="xw", bufs=1) as xw,
            tc.tile_pool(name="hp", bufs=2, space="PSUM") as hp,
        ):
            xT_sb = xw.tile([128, TPAD], bf16)
            nc.sync.dma_start(xT_sb[:], xT_d[:, :])
            for t in range(NT):
                ps = hp.tile([128, 128], f32)
                nc.tensor.matmul(ps[:], lhsT=xT_sb[:, t * 128:(t + 1) * 128],
                                 rhs=w1_sb[:], start=True, stop=True)
                nc.scalar.activation(h_loc[:, t * 128:(t + 1) * 128], ps[:],
                                     AF.Copy, scale=disc_sb[:, t:t + 1])
                rows = min(128, NV - t * 128)
                nc.sync.dma_start(h_own[t * 128:t * 128 + rows, :],
                                  h_loc[0:rows, t * 128:(t + 1) * 128])

        nc.gpsimd.collective_compute(
            "AllGather", ALU.bypass, replica_groups=[list(range(P))],
            ins=[h_own[:].opt()], outs=[h_full[:].opt()])

        # phase-C constants: loads overlap the AllGather
        iota_sb = cpool.tile([128, 128], bf16)
        nc.sync.dma_start(iota_sb[:], iota_d[:, :])
        eye_sb = cpool.tile([16, 16], f32)
        nc.sync.dma_start(eye_sb[:], eye_d[:, :])
        idxs_sb = cpool.tile([128, NIDX // 16], i16)
        nc.sync.dma_start(idxs_sb[:], idxs_d[:, :])
        dloc_sb = cpool.tile([128, NCH], bf16)
        nc.sync.dma_start(dloc_sb[:], dloc_d[:, :])
        eye128_sb = cpool.tile([128, 128], bf16)
        nc.sync.dma_start(eye128_sb[:], eye128_d[:, :])
        w2_sb = cpool.tile([HID, C], f32)
        nc.sync.dma_start(w2_sb[:], w2_d[:, :])
        if use_b1:
            rdis_sb = cpool.tile([1, TPAD], f32)
            nc.sync.dma_start(rdis_sb[:], rdis_d[:, :])
            b1_sb = cpool.tile([1, F], f32)
            nc.sync.dma_start(b1_sb[:], b1_d[:, :])
        if use_b2:
            b2_sb = cpool.tile([1, C], f32)
            nc.sync.dma_start(b2_sb[:], b2_d[:, :])

        # ---------------- phase C: layer-1 aggregation + layer-2 contraction
        with tc.tile_pool(name="ptp", bufs=1, space="PSUM") as ptp:
            poolT = ptp.tile([128, GP], f32)
            i_l2 = 0
            with (
                tc.tile_pool(name="glo", bufs=NQ) as glo_p,
                tc.tile_pool(name="ghi", bufs=NQ) as ghi_p,
                tc.tile_pool(name="selp", bufs=3) as selp,
                tc.tile_pool(name="qp", bufs=3) as qp,
                tc.tile_pool(name="aggp", bufs=7, space="PSUM") as aggp,
            ):
                for b in range(NB):
                    m = plan["batch_meta"][b]
                    nlo, nhi = m["nlo"], m["nhi"]
                    ngb = int(os.environ.get("GCN_NGB", str(NB)))
                    glo = ghi = None
                    if b >= ngb:
                        continue
                    if nlo and stage >= 2:
                        glo = glo_p.tile([128, nlo, 128], bf16, tag="glo")
                        nc.gpsimd.dma_gather(
                            out_ap=glo[:], in_ap=h_full[0:HALF, :],
                            idxs_ap=idxs_sb[:, m["icol_lo"]:
                                            m["icol_lo"] + nlo * 8],
                            num_idxs=nlo * 128, num_idxs_reg=nlo * 128,
                            elem_size=F, single_packet=False,
                            queue_num=(2 * b) % NQ)
                    if nhi and stage >= 2:
                        ghi = ghi_p.tile([128, nhi, 128], bf16, tag="ghi")
                        nc.gpsimd.dma_gather(
                            out_ap=ghi[:], in_ap=h_full[HALF:N, :],
                            idxs_ap=idxs_sb[:, m["icol_hi"]:
                                            m["icol_hi"] + nhi * 8],
                            num_idxs=nhi * 128, num_idxs_reg=nhi * 128,
                            elem_size=F, single_packet=False,
                            queue_num=(2 * b + 1) % NQ)
                    if stage < 3:
                        if stage == 2 and (glo is not None or ghi is not None):
                            junk = selp.tile([128, 128], bf16, tag="sel")
                            gj = glo if glo is not None else ghi
                            nc.vector.tensor_copy(junk[:], gj[:, 0, :])
                            nc.sync.dma_start(ar_in[0:128, 0:64],
                                              junk[:, 0:64])
                        continue
                    for t in plan["tiles_of_batch"][b]:
                        tm = plan["tile_meta"][t]
                        t_nlo, t_nhi = tm["nlo"], tm["nhi"]
                        nch_t = t_nlo + t_nhi
                        # one is_eq builds all selectors for this tile
                        sel = selp.tile([128, nch_t, 128], bf16, tag="sel")
                        nc.vector.tensor_tensor(
                            out=sel[:],
                            in0=iota_sb[:, None, :].to_broadcast(
                                [128, nch_t, 128]),
                            in1=dloc_sb[:, tm["col0"]:tm["col0"] + nch_t,
                                        None].to_broadcast([128, nch_t, 128]),
                            op=ALU.is_equal)
                        ps = aggp.tile([128, 128], f32, tag="agg")
                        # self-loop: the message is h_loc itself; identity
                        # lhsT adds it into the PSUM accumulation
                        nc.tensor.matmul(
                            ps[:], lhsT=eye128_sb[:],
                            rhs=h_loc[:, t * 128:(t + 1) * 128],
                            start=True, stop=False)
                        if use_b1:
                            nc.tensor.matmul(
                                ps[:], lhsT=rdis_sb[0:1, t * 128:(t + 1) * 128],
                                rhs=b1_sb[:], start=False, stop=False)
                        for ci in range(nch_t):
                            if ci < t_nlo:
                                gsrc, joff = glo, tm["jlo"] + ci
                            else:
                                gsrc, joff = ghi, tm["jhi"] + (ci - t_nlo)
                            nc.tensor.matmul(
                                ps[:], lhsT=sel[:, ci, :],
                                rhs=gsrc[:, joff, :],
                                start=False, stop=(ci == nch_t - 1))
                        nc.scalar.activation(
                            h1_sb[:, t * 128:(t + 1) * 128], ps[:], AF.Relu,
                            scale=disc_sb[:, t:t + 1])
                        # layer 2: poolT += H1_tile^T-contraction with Q block
                        qt = qp.tile([128, GP], bf16, tag="q")
                        nc.sync.dma_start(
                            qt[:], qb_d[t * 128:(t + 1) * 128, :])
                        nc.tensor.matmul(
                            poolT[:],
                            lhsT=h1_sb[:, t * 128:(t + 1) * 128],
                            rhs=qt[:],
                            start=(i_l2 == 0), stop=(i_l2 == NT - 1))
                        i_l2 += 1

            pt_sb = cpool.tile([128, GP], f32)
            if stage >= 3:
                nc.scalar.activation(pt_sb[:], poolT[:], AF.Copy)
            else:
                nc.vector.memset(pt_sb[:], 0.0)
            nc.sync.dma_start(ar_in[:], pt_sb[:])

        nc.gpsimd.collective_compute(
            "AllReduce", ALU.add, replica_groups=[list(range(P))],
            ins=[ar_in[:].opt()], outs=[ar_out[:].opt()])

        # ---------------- phase D: W2, bias, log_softmax
        with (
            tc.tile_pool(name="fin", bufs=1) as fin,
            tc.tile_pool(name="fps", bufs=2, space="PSUM") as fps,
            tc.tile_pool(name="sm", bufs=4) as smp,
        ):
            pooledT = fin.tile([128, GP], f32)
            nc.sync.dma_start(pooledT[:], ar_out[:])
            out2 = fps.tile([16, GP], f32, tag="out2")
            nc.tensor.matmul(out2[:], lhsT=w2_sb[:], rhs=pooledT[:],
                             start=True, stop=not use_b2)
            if use_b2:
                ones = fin.tile([1, GP], f32)
                nc.vector.memset(ones[:], 1.0)
                nc.tensor.matmul(out2[:], lhsT=b2_sb[:], rhs=ones[:],
                                 start=False, stop=True)
            logitsT = fin.tile([16, GP], f32)
            nc.scalar.activation(logitsT[:], out2[:], AF.Copy)
            for gt in range(min(GT, -(-G // 128))):
                tp = fps.tile([128, 16], f32, tag="tp")
                nc.tensor.transpose(
                    tp[:], logitsT[:, gt * 128:(gt + 1) * 128], eye_sb[:])
                nmx = smp.tile([128, 1], f32, tag="nmx")
                nc.vector.reduce_max(out=nmx[:], in_=tp[:],
                                     axis=mybir.AxisListType.X, negate=True)
                ex = smp.tile([128, 16], f32, tag="ex")
                nc.scalar.activation(ex[:], tp[:], AF.Exp, bias=nmx[:, 0:1])
                sm = smp.tile([128, 1], f32, tag="sm")
                nc.vector.reduce_sum(out=sm[:], in_=ex[:],
                                     axis=mybir.AxisListType.X)
                lse = smp.tile([128, 1], f32, tag="lse")
                nc.scalar.activation(lse[:], sm[:], AF.Ln)
                res = smp.tile([128, 16], f32, tag="res")
                nc.vector.tensor_scalar(res[:], tp[:], nmx[:, 0:1],
                                        lse[:, 0:1], ALU.add, ALU.subtract)
                rows = min(128, G - gt * 128)
                nc.sync.dma_start(y_d[gt * 128:gt * 128 + rows, :],
                                  res[0:rows, :])
        dram.release()
        cpool.release()
    nc.compile()
    return nc


# ---------------------------------------------------------------- entry
def kernel(x, W1, b1, W2, b2, edge_src, edge_dst, batch):
    global LAST_EXEC_NS, LAST_RESULT
    plan, in_maps = _preprocess(x, W1, b1, W2, b2,
                                edge_src, edge_dst, batch)
    nc = _build(plan)
    trace = bool(int(os.environ.get("GCN_TRACE", "0")))
    kw = {}
    if trace and _install_profile_hook():
        kw = dict(trace=True, trace_cores=[0])
    res = run_bass_kernel_spmd(nc, in_maps, core_ids=list(range(P)), **kw)
    LAST_RESULT = res
    LAST_EXEC_NS = res.exec_time_ns
    return np.ascontiguousarray(res.results[0]["y"].astype(np.float32))


# revision 6
# speedup vs baseline: 2.2004x; 1.0852x over previous
"""GCN graph-classification kernel for 8 Trainium2 NeuronCores.

Model (PyG-style GCNConv x2 + mean pool + log_softmax):
    h   = x @ W1
    H1  = relu(Ahat @ h + b1)          Ahat = D^-1/2 (A + I) D^-1/2
    H2  = Ahat @ (H1 @ W2) + b2
    out = log_softmax(mean-pool-per-graph(H2))

Distribution strategy (8 cores):
  * nodes partitioned contiguously (6250/core); per-core in-degree-sorted
    permutation so destination tiles have homogeneous degrees.
  * layer 1: h computed locally (bf16), dis-prescaled, AllGathered in TWO
    pipelined Shared-output collectives (tiles 0-24 -> table A of 25600
    rows, tiles 25-48 -> table B of 24576 rows; both fit the int16 gather
    index range, so no lo/hi split is needed and the A-gathers start as
    soon as AG1 lands, while the second half of phase B still runs).
    Each core aggregates its own nodes' in-edges with dma_gather (256B
    bf16 edge messages) + one-hot selector matmuls accumulating in PSUM.
    Gathers are spread round-robin across 4 SWDGE queues so descriptor
    generation runs on 4 Q7 core-pairs in parallel. Self-loops are folded
    into one identity-matmul per tile from the locally kept h tiles.
  * layer 2 + pooling folded:  pooled = (Q @ H1) @ W2 + b2  with
    Q = P_mean @ Ahat  (500 x 50000, built dense-per-node-tile on host).
    Each core contracts its own H1 tiles against its Q blocks -> partial
    per-graph sums -> AllReduce (500x128 floats) -> W2 -> log_softmax.
  All symmetric-norm factors, mean-pool counts and the permutation are
  folded into host-built index/selector/Q arrays (pure index-side prep).
"""

import os
import numpy as np

import concourse.bacc as bacc
import concourse.mybir as mybir
from concourse import tile
from concourse.bass_utils import run_bass_kernel_spmd

# ---------------------------------------------------------------- constants
N, E, F, HID, C, G = 50000, 600000, 128, 128, 16, 500
P = 8                      # NeuronCores
NV = N // P                # nodes per core
NT = (NV + 127) // 128     # node tiles per core (49)
TPAD = NT * 128            # padded per-core node count (6272)
GP = 512                   # padded graph count
GT = GP // 128             # graph tiles
NTA = 25                   # tiles in table A
ROWS_A = NTA * 128         # 3200 rows/core in table A (P*3200 = 25600)
ROWS_B = TPAD - ROWS_A     # 3072 rows/core in table B (P*3072 = 24576)
NB = 7                     # layer-1 gather batches
NQ = 4                     # SWDGE queues (parallel gather descriptor gen)

AF = mybir.ActivationFunctionType
ALU = mybir.AluOpType

LAST_EXEC_NS = None
LAST_RESULT = None


def _install_profile_hook():
    """The agent image's antenv lacks axon_hooks; shim it so
    run_bass_kernel_spmd(trace=True) can capture NTFF profiles."""
    import sys
    import types
    if "antenv.axon_hooks" in sys.modules:
        return True
    try:
        from trn_agent_boot.trn_boot import _ntff_profile_via_ctypes
        hook = _ntff_profile_via_ctypes("/opt/axon/libaxon_pjrt.so")
        if hook is None:
            return False
        mod = types.ModuleType("antenv.axon_hooks")
        mod._hook = hook
        mod.get_axon_ntff_profile_hook = lambda: mod._hook

        def _set(h):
            mod._hook = h
        mod.set_axon_ntff_profile_hook = _set
        sys.modules["antenv.axon_hooks"] = mod
        import antenv
        antenv.axon_hooks = mod
        return True
    except Exception as e:  # profiling is best-effort
        print(f"profile hook unavailable: {e}")
        return False


# ---------------------------------------------------------------- host prep
def _preprocess(x, W1, b1, W2, b2, edge_src, edge_dst, batch):
    import ml_dtypes
    f32 = np.float32
    bf16 = ml_dtypes.bfloat16
    src = np.asarray(edge_src, np.int64)
    dst = np.asarray(edge_dst, np.int64)
    bat = np.asarray(batch, np.int64)
    x = np.asarray(x, f32)

    deg = np.bincount(dst, minlength=N).astype(np.float64) + 1.0
    dis = 1.0 / np.sqrt(deg)
    cnt = np.maximum(np.bincount(bat, minlength=G), 1).astype(np.float64)

    # per-core degree-descending node permutation
    pos = np.empty(N, np.int64)
    order = np.empty(N, np.int64)      # order[k*NV+j] = node at position j
    for k in range(P):
        v0 = k * NV
        loc = np.argsort(-deg[v0:v0 + NV], kind="stable")
        order[v0:v0 + NV] = v0 + loc
        pos[v0 + loc] = np.arange(NV)

    # ---- layer-1 gather edges (no self-loops; those come from local h
    # tiles via an identity matmul), grouped (core, tile, src-table)
    d_own = dst // NV
    d_pos = pos[dst]
    t_of = d_pos // 128
    dloc_v = (d_pos % 128).astype(f32)
    s_core = src // NV
    s_pos = pos[src]
    is_B = (s_pos >= ROWS_A).astype(np.int64)
    idx_v = np.where(is_B, s_core * ROWS_B + (s_pos - ROWS_A),
                     s_core * ROWS_A + s_pos).astype(np.int16)

    key = (d_own * NT + t_of) * 2 + is_B
    ordr = np.argsort(key, kind="stable")
    idx_s = idx_v[ordr]
    dloc_s = dloc_v[ordr]
    bounds = np.searchsorted(key[ordr], np.arange(P * NT * 2 + 1))
    cnts = np.diff(bounds).reshape(P, NT, 2)
    CH = -(-cnts // 128)               # chunks per (core, tile, table)
    CH = CH.max(axis=0)                # [NT, 2]  uniform across cores

    # batches: stride-interleaved tiles so per-batch work is balanced
    tiles_of_batch = [[t for t in range(NT) if t % NB == b] for b in range(NB)]

    # chunk-column layout: per batch, chunks grouped PER TILE (A then B)
    # so each tile's selector build is one contiguous is_eq op.
    # gather-index layout: per batch, [A chunks tile-major][B chunks
    # tile-major] (matches the two dma_gather calls).
    batch_meta = []        # per batch: dict(nA, nB, icol_A, icol_B)
    tile_meta = {}         # per tile: dict(col0, nA, nB, jA, jB)
    col = 0
    icol = 0
    for b in range(NB):
        nA = int(sum(CH[t, 0] for t in tiles_of_batch[b]))
        nB = int(sum(CH[t, 1] for t in tiles_of_batch[b]))
        batch_meta.append(dict(nA=nA, nB=nB,
                               icol_A=icol, icol_B=icol + nA * 8))
        jA = 0
        jB = 0
        for t in tiles_of_batch[b]:
            tile_meta[t] = dict(col0=col, nA=int(CH[t, 0]), nB=int(CH[t, 1]),
                                jA=jA, jB=jB)
            col += int(CH[t, 0]) + int(CH[t, 1])
            jA += int(CH[t, 0])
            jB += int(CH[t, 1])
        icol += (nA + nB) * 8
    NCH = col
    NIDX = NCH * 128

    # per-core data arrays
    xT = np.zeros((P, 128, TPAD), bf16)
    disc = np.zeros((P, 128, NT), f32)
    qb = np.zeros((P, TPAD, GP), f32)
    dloc_all = np.full((P, 128, NCH), -1.0, bf16)
    idx_flat = np.zeros((P, NIDX), np.int16)

    for k in range(P):
        ok = order[k * NV:(k + 1) * NV]
        xT[k, :, :NV] = x[ok].T.astype(bf16)
        d = np.zeros(TPAD, f32)
        d[:NV] = dis[ok].astype(f32)
        disc[k] = d.reshape(NT, 128).T

    # fill chunk idx / dloc tables (idx layout: per batch, A tile-major
    # then B tile-major; dloc layout: per batch, per tile A then B)
    for b in range(NB):
        m = batch_meta[b]
        for h, base_icol in ((0, m["icol_A"]), (1, m["icol_B"])):
            jh = 0
            for t in tiles_of_batch[b]:
                nchunk = int(CH[t, h])
                if nchunk > 0:
                    tm = tile_meta[t]
                    for k in range(P):
                        gi = (k * NT + t) * 2 + h
                        g0, g1 = bounds[gi], bounds[gi + 1]
                        n = g1 - g0
                        fbase = base_icol * 16 + jh * 128
                        idx_flat[k, fbase:fbase + n] = idx_s[g0:g1]
                        pp = np.arange(n) % 128
                        cc = np.arange(n) // 128
                        colbase = tm["col0"] + (0 if h == 0 else tm["nA"])
                        dloc_all[k, pp, colbase + cc] = \
                            dloc_s[g0:g1].astype(bf16)
                jh += nchunk
    assert idx_flat.min() >= 0
    assert int(idx_flat.reshape(-1).max()) < P * ROWS_A
    # wrap gather indices: i -> [i % 16, i // 16], replicated to 128 partitions
    idxs = np.tile(
        idx_flat.reshape(P, NIDX // 16, 16).transpose(0, 2, 1), (1, 8, 1)
    ).astype(np.int16)

    # ---- layer-2 Q blocks: qb[core, pos[src], g] += norm/cnt[g]
    # (self-loops included here)
    e_src = np.concatenate([src, np.arange(N)])
    e_dst = np.concatenate([dst, np.arange(N)])
    g_of = bat[e_dst]
    val = (dis[e_src] * dis[e_dst] / cnt[g_of]).astype(f32)
    np.add.at(qb, (e_src // NV, pos[e_src], g_of), val)
    qb = qb.astype(bf16)

    iota_bf = np.broadcast_to(
        np.arange(128, dtype=bf16), (128, 128)).copy()
    eye16 = np.eye(16, dtype=f32)
    eye128 = np.eye(128, dtype=bf16)
    widx = np.zeros((128, 8), np.int16)

    W1 = np.ascontiguousarray(np.asarray(W1, f32).astype(bf16))
    W2 = np.ascontiguousarray(np.asarray(W2, f32))
    b1 = np.asarray(b1, f32)
    b2 = np.asarray(b2, f32)
    use_b1 = bool(np.any(b1))
    use_b2 = bool(np.any(b2))

    in_maps = []
    for k in range(P):
        m = {
            "xT": np.ascontiguousarray(xT[k]),
            "qb": np.ascontiguousarray(qb[k]),
            "idxs": np.ascontiguousarray(idxs[k]),
            "dloc": np.ascontiguousarray(dloc_all[k]),
            "disc": np.ascontiguousarray(disc[k]),
            "eye128": eye128,
            "w1": W1, "w2": W2,
            "iota": iota_bf, "eye16": eye16, "widx": widx,
        }
        if use_b1:
            rr = np.zeros((1, TPAD), f32)
            rr[0, :NV] = np.sqrt(deg[order[k * NV:(k + 1) * NV]]).astype(f32)
            m["rdis"] = rr
            m["b1r"] = b1.reshape(1, F)
        if use_b2:
            m["b2r"] = b2.reshape(1, C)
        in_maps.append(m)

    plan = dict(NCH=NCH, NIDX=NIDX, CH=CH, tiles_of_batch=tiles_of_batch,
                batch_meta=batch_meta, tile_meta=tile_meta,
                use_b1=use_b1, use_b2=use_b2)
    return plan, in_maps


# ---------------------------------------------------------------- bass build
def _build(plan):
    dt = mybir.dt
    f32, bf16, i16 = dt.float32, dt.bfloat16, dt.int16
    NCH, NIDX = plan["NCH"], plan["NIDX"]
    use_b1, use_b2 = plan["use_b1"], plan["use_b2"]
    single_packet = bool(int(os.environ.get("GCN_SP", "0")))

    stage = int(os.environ.get("GCN_STAGE", "3"))  # 1: no phase C; 2: +gathers
    nc = bacc.Bacc("TRN2", target_bir_lowering=False, debug=False,
                   num_devices=P, num_swdge_queues=NQ)
    xT_d = nc.dram_tensor("xT", [128, TPAD], bf16, kind="ExternalInput")
    qb_d = nc.dram_tensor("qb", [TPAD, GP], bf16, kind="ExternalInput")
    idxs_d = nc.dram_tensor("idxs", [128, NIDX // 16], i16, kind="ExternalInput")
    dloc_d = nc.dram_tensor("dloc", [128, NCH], bf16, kind="ExternalInput")
    disc_d = nc.dram_tensor("disc", [128, NT], f32, kind="ExternalInput")
    eye128_d = nc.dram_tensor("eye128", [128, 128], bf16, kind="ExternalInput")
    w1_d = nc.dram_tensor("w1", [F, HID], bf16, kind="ExternalInput")
    w2_d = nc.dram_tensor("w2", [HID, C], f32, kind="ExternalInput")
    iota_d = nc.dram_tensor("iota", [128, 128], bf16, kind="ExternalInput")
    eye_d = nc.dram_tensor("eye16", [16, 16], f32, kind="ExternalInput")
    widx_d = nc.dram_tensor("widx", [128, 8], i16, kind="ExternalInput")
    if use_b1:
        rdis_d = nc.dram_tensor("rdis", [1, TPAD], f32, kind="ExternalInput")
        b1_d = nc.dram_tensor("b1r", [1, F], f32, kind="ExternalInput")
    if use_b2:
        b2_d = nc.dram_tensor("b2r", [1, C], f32, kind="ExternalInput")
    y_d = nc.dram_tensor("y", [G, C], f32, kind="ExternalOutput")

    with tile.TileContext(nc) as tc:
        cpool = tc.alloc_tile_pool(name="const", bufs=1)
        dram = tc.alloc_tile_pool(name="dram", bufs=1, space="DRAM")

        h_ownA = dram.tile([ROWS_A, F], bf16)
        h_ownB = dram.tile([ROWS_B, F], bf16)
        h_fullA = dram.tile([P * ROWS_A, F], bf16, addr_space="Shared")
        h_fullB = dram.tile([P * ROWS_B, F], bf16, addr_space="Shared")
        ar_in = dram.tile([128, GP], f32)
        ar_out = dram.tile([128, GP], f32)

        # warm gather: preloads the Q7 ext-isa library (~9us) before it's
        # needed; reads a fixed xT row, result unused.
        widx_sb = cpool.tile([128, 8], i16)
        nc.sync.dma_start(widx_sb[:], widx_d[:, :])
        warm_sb = cpool.tile([128, 1, 128], bf16)
        if int(os.environ.get("GCN_WARM", "1")):
            nc.gpsimd.dma_gather(
                out_ap=warm_sb[:], in_ap=xT_d[:, 0:128],
                idxs_ap=widx_sb[:, :], num_idxs=128, num_idxs_reg=128,
                elem_size=F, elem_step=TPAD, single_packet=False,
                queue_num=0)

        # phase-B-critical constants first so their DMAs run first
        w1_sb = cpool.tile([F, HID], bf16)
        nc.sync.dma_start(w1_sb[:], w1_d[:, :])
        disc_sb = cpool.tile([128, NT], f32)
        nc.sync.dma_start(disc_sb[:], disc_d[:, :])
        h_loc = cpool.tile([128, TPAD], bf16)   # local dis*h, node-major tiles
        h1_sb = cpool.tile([128, TPAD], bf16)
        idxs_sb = cpool.tile([128, NIDX // 16], i16)
        nc.scalar.dma_start(idxs_sb[:], idxs_d[:, :])
        dloc_sb = cpool.tile([128, NCH], bf16)
        nc.scalar.dma_start(dloc_sb[:], dloc_d[:, :])

        # ---------------- phase B: h = dis * (x @ W1), 2 AllGathers
        with (
            tc.tile_pool(name="xw", bufs=1) as xw,
            tc.tile_pool(name="hp", bufs=2, space="PSUM") as hp,
        ):
            xT_sb = xw.tile([128, TPAD], bf16)
            nc.sync.dma_start(xT_sb[:], xT_d[:, :])
            for t in range(NT):
                ps = hp.tile([128, 128], f32)
                nc.tensor.matmul(ps[:], lhsT=xT_sb[:, t * 128:(t + 1) * 128],
                                 rhs=w1_sb[:], start=True, stop=True)
                nc.scalar.activation(h_loc[:, t * 128:(t + 1) * 128], ps[:],
                                     AF.Copy, scale=disc_sb[:, t:t + 1])
                eng = nc.sync if t % 2 == 0 else nc.scalar
                if t < NTA:
                    eng.dma_start(h_ownA[t * 128:(t + 1) * 128, :],
                                  h_loc[:, t * 128:(t + 1) * 128])
                else:
                    tb = t - NTA
                    eng.dma_start(h_ownB[tb * 128:(tb + 1) * 128, :],
                                  h_loc[:, t * 128:(t + 1) * 128])
                if t == NTA - 1:
                    nc.gpsimd.collective_compute(
                        "AllGather", ALU.bypass,
                        replica_groups=[list(range(P))],
                        ins=[h_ownA[:].opt()], outs=[h_fullA[:].opt()])
            nc.gpsimd.collective_compute(
                "AllGather", ALU.bypass, replica_groups=[list(range(P))],
                ins=[h_ownB[:].opt()], outs=[h_fullB[:].opt()])

        # remaining phase-C constants
        iota_sb = cpool.tile([128, 128], bf16)
        nc.sync.dma_start(iota_sb[:], iota_d[:, :])
        eye_sb = cpool.tile([16, 16], f32)
        nc.sync.dma_start(eye_sb[:], eye_d[:, :])
        eye128_sb = cpool.tile([128, 128], bf16)
        nc.sync.dma_start(eye128_sb[:], eye128_d[:, :])
        w2_sb = cpool.tile([HID, C], f32)
        nc.sync.dma_start(w2_sb[:], w2_d[:, :])
        if use_b1:
            rdis_sb = cpool.tile([1, TPAD], f32)
            nc.sync.dma_start(rdis_sb[:], rdis_d[:, :])
            b1_sb = cpool.tile([1, F], f32)
            nc.sync.dma_start(b1_sb[:], b1_d[:, :])
        if use_b2:
            b2_sb = cpool.tile([1, C], f32)
            nc.sync.dma_start(b2_sb[:], b2_d[:, :])

        # ---------------- phase C: layer-1 aggregation + layer-2 contraction
        with tc.tile_pool(name="ptp", bufs=1, space="PSUM") as ptp:
            poolT = ptp.tile([128, GP], f32)
            i_l2 = 0
            with (
                tc.tile_pool(name="ga", bufs=4) as ga_p,
                tc.tile_pool(name="gb", bufs=4) as gb_p,
                tc.tile_pool(name="selp", bufs=6) as selp,
                tc.tile_pool(name="qp", bufs=3) as qp,
                tc.tile_pool(name="aggp", bufs=7, space="PSUM") as aggp,
            ):
                for b in range(NB):
                    m = plan["batch_meta"][b]
                    nA, nB = m["nA"], m["nB"]
                    ngb = int(os.environ.get("GCN_NGB", str(NB)))
                    gA = gB = None
                    if b >= ngb:
                        continue
                    if nA and stage >= 2:
                        gA = ga_p.tile([128, nA, 128], bf16, tag="ga")
                        nc.gpsimd.dma_gather(
                            out_ap=gA[:], in_ap=h_fullA[:, :],
                            idxs_ap=idxs_sb[:, m["icol_A"]:
                                            m["icol_A"] + nA * 8],
                            num_idxs=nA * 128, num_idxs_reg=nA * 128,
                            elem_size=F, single_packet=single_packet,
                            queue_num=(2 * b) % NQ)
                    if nB and stage >= 2:
                        gB = gb_p.tile([128, nB, 128], bf16, tag="gb")
                        nc.gpsimd.dma_gather(
                            out_ap=gB[:], in_ap=h_fullB[:, :],
                            idxs_ap=idxs_sb[:, m["icol_B"]:
                                            m["icol_B"] + nB * 8],
                            num_idxs=nB * 128, num_idxs_reg=nB * 128,
                            elem_size=F, single_packet=single_packet,
                            queue_num=(2 * b + 1) % NQ)
                    if stage < 3:
                        if stage == 2 and (gA is not None or gB is not None):
                            junk = selp.tile([128, 128], bf16, tag="sel")
                            gj = gA if gA is not None else gB
                            nc.vector.tensor_copy(junk[:], gj[:, 0, :])
                            nc.sync.dma_start(ar_in[0:128, 0:64],
                                              junk[:, 0:64])
                        continue
                    for t in plan["tiles_of_batch"][b]:
                        tm = plan["tile_meta"][t]
                        t_nA, t_nB = tm["nA"], tm["nB"]
                        nch_t = t_nA + t_nB
                        # one is_eq builds all selectors for this tile
                        sel = selp.tile([128, nch_t, 128], bf16, tag="sel")
                        nc.vector.tensor_tensor(
                            out=sel[:],
                            in0=iota_sb[:, None, :].to_broadcast(
                                [128, nch_t, 128]),
                            in1=dloc_sb[:, tm["col0"]:tm["col0"] + nch_t,
                                        None].to_broadcast([128, nch_t, 128]),
                            op=ALU.is_equal)
                        ps = aggp.tile([128, 128], f32, tag="agg")
                        # self-loop: the message is h_loc itself; identity
                        # lhsT adds it into the PSUM accumulation
                        nc.tensor.matmul(
                            ps[:], lhsT=eye128_sb[:],
                            rhs=h_loc[:, t * 128:(t + 1) * 128],
                            start=True, stop=False)
                        if use_b1:
                            nc.tensor.matmul(
                                ps[:], lhsT=rdis_sb[0:1, t * 128:(t + 1) * 128],
                                rhs=b1_sb[:], start=False, stop=False)
                        for ci in range(nch_t):
                            if ci < t_nA:
                                gsrc, joff = gA, tm["jA"] + ci
                            else:
                                gsrc, joff = gB, tm["jB"] + (ci - t_nA)
                            nc.tensor.matmul(
                                ps[:], lhsT=sel[:, ci, :],
                                rhs=gsrc[:, joff, :],
                                start=False, stop=(ci == nch_t - 1))
                        nc.scalar.activation(
                            h1_sb[:, t * 128:(t + 1) * 128], ps[:], AF.Relu,
                            scale=disc_sb[:, t:t + 1])
                        # layer 2: poolT += H1_tile^T-contraction with Q block
                        qt = qp.tile([128, GP], bf16, tag="q")
                        nc.sync.dma_start(
                            qt[:], qb_d[t * 128:(t + 1) * 128, :])
                        nc.tensor.matmul(
                            poolT[:],
                            lhsT=h1_sb[:, t * 128:(t + 1) * 128],
                            rhs=qt[:],
                            start=(i_l2 == 0), stop=(i_l2 == NT - 1))
                        i_l2 += 1

            pt_sb = cpool.tile([128, GP], f32)
            if stage >= 3:
                nc.scalar.activation(pt_sb[:], poolT[:], AF.Copy)
            else:
                nc.vector.memset(pt_sb[:], 0.0)
            nc.sync.dma_start(ar_in[:], pt_sb[:])

        nc.gpsimd.collective_compute(
            "AllReduce", ALU.add, replica_groups=[list(range(P))],
            ins=[ar_in[:].opt()], outs=[ar_out[:].opt()])

        # ---------------- phase D: W2, bias, log_softmax
        with (
            tc.tile_pool(name="fin", bufs=1) as fin,
            tc.tile_pool(name="fps", bufs=2, space="PSUM") as fps,
            tc.tile_pool(name="sm", bufs=4) as smp,
        ):
            pooledT = fin.tile([128, GP], f32)
            nc.sync.dma_start(pooledT[:], ar_out[:])
            out2 = fps.tile([16, GP], f32, tag="out2")
            nc.tensor.matmul(out2[:], lhsT=w2_sb[:], rhs=pooledT[:],
                             start=True, stop=not use_b2)
            if use_b2:
                ones = fin.tile([1, GP], f32)
                nc.vector.memset(ones[:], 1.0)
                nc.tensor.matmul(out2[:], lhsT=b2_sb[:], rhs=ones[:],
                                 start=False, stop=True)
            logitsT = fin.tile([16, GP], f32)
            nc.scalar.activation(logitsT[:], out2[:], AF.Copy)
            for gt in range(min(GT, -(-G // 128))):
                tp = fps.tile([128, 16], f32, tag="tp")
                nc.tensor.transpose(
                    tp[:], logitsT[:, gt * 128:(gt + 1) * 128], eye_sb[:])
                nmx = smp.tile([128, 1], f32, tag="nmx")
                nc.vector.reduce_max(out=nmx[:], in_=tp[:],
                                     axis=mybir.AxisListType.X, negate=True)
                ex = smp.tile([128, 16], f32, tag="ex")
                nc.scalar.activation(ex[:], tp[:], AF.Exp, bias=nmx[:, 0:1])
                sm = smp.tile([128, 1], f32, tag="sm")
                nc.vector.reduce_sum(out=sm[:], in_=ex[:],
                                     axis=mybir.AxisListType.X)
                lse = smp.tile([128, 1], f32, tag="lse")
                nc.scalar.activation(lse[:], sm[:], AF.Ln)
                res = smp.tile([128, 16], f32, tag="res")
                nc.vector.tensor_scalar(res[:], tp[:], nmx[:, 0:1],
                                        lse[:, 0:1], ALU.add, ALU.subtract)
                rows = min(128, G - gt * 128)
                nc.sync.dma_start(y_d[gt * 128:gt * 128 + rows, :],
                                  res[0:rows, :])
        dram.release()
        cpool.release()
    nc.compile()
    return nc


# ---------------------------------------------------------------- entry
def kernel(x, W1, b1, W2, b2, edge_src, edge_dst, batch):
    global LAST_EXEC_NS, LAST_RESULT
    plan, in_maps = _preprocess(x, W1, b1, W2, b2,
                                edge_src, edge_dst, batch)
    nc = _build(plan)
    trace = bool(int(os.environ.get("GCN_TRACE", "0")))
    kw = {}
    if trace and _install_profile_hook():
        kw = dict(trace=True, trace_cores=[0])
    res = run_bass_kernel_spmd(nc, in_maps, core_ids=list(range(P)), **kw)
    LAST_RESULT = res
    LAST_EXEC_NS = res.exec_time_ns
    return np.ascontiguousarray(res.results[0]["y"].astype(np.float32))


# revision 7
# speedup vs baseline: 2.3276x; 1.0578x over previous
"""GCN graph-classification kernel for 8 Trainium2 NeuronCores.

Model (PyG-style GCNConv x2 + mean pool + log_softmax):
    h   = x @ W1
    H1  = relu(Ahat @ h + b1)          Ahat = D^-1/2 (A + I) D^-1/2
    H2  = Ahat @ (H1 @ W2) + b2
    out = log_softmax(mean-pool-per-graph(H2))

Distribution strategy (8 cores):
  * nodes partitioned contiguously (6250/core); per-core in-degree-sorted
    permutation so destination tiles have homogeneous degrees.
  * layer 1: h computed locally (bf16), dis-prescaled, AllGathered in TWO
    pipelined Shared-output collectives (tiles 0-24 -> table A of 25600
    rows, tiles 25-48 -> table B of 24576 rows; both fit the int16 gather
    index range, so no lo/hi split is needed and the A-gathers start as
    soon as AG1 lands, while the second half of phase B still runs).
    Each core aggregates its own nodes' in-edges with dma_gather (256B
    bf16 edge messages) + one-hot selector matmuls accumulating in PSUM.
    Gathers are spread round-robin across 4 SWDGE queues so descriptor
    generation runs on 4 Q7 core-pairs in parallel. Self-loops are folded
    into one identity-matmul per tile from the locally kept h tiles.
  * layer 2 + pooling folded:  pooled = (Q @ H1) @ W2 + b2  with
    Q = P_mean @ Ahat  (500 x 50000, built dense-per-node-tile on host).
    Each core contracts its own H1 tiles against its Q blocks -> partial
    per-graph sums -> AllReduce (500x128 floats) -> W2 -> log_softmax.
  All symmetric-norm factors, mean-pool counts and the permutation are
  folded into host-built index/selector/Q arrays (pure index-side prep).
"""

import os
import numpy as np

import concourse.bacc as bacc
import concourse.mybir as mybir
from concourse import tile
from concourse.bass_utils import run_bass_kernel_spmd

# ---------------------------------------------------------------- constants
N, E, F, HID, C, G = 50000, 600000, 128, 128, 16, 500
P = 8                      # NeuronCores
NV = N // P                # nodes per core
NT = (NV + 127) // 128     # node tiles per core (49)
TPAD = NT * 128            # padded per-core node count (6272)
GP = 512                   # padded graph count
GT = GP // 128             # graph tiles
NTA = 25                   # tiles in table A
ROWS_A = NTA * 128         # 3200 rows/core in table A (P*3200 = 25600)
ROWS_B = TPAD - ROWS_A     # 3072 rows/core in table B (P*3072 = 24576)
NB = 10                    # layer-1 gather batches
NQ = 4                     # SWDGE queues (parallel gather descriptor gen)

AF = mybir.ActivationFunctionType
ALU = mybir.AluOpType

LAST_EXEC_NS = None
LAST_RESULT = None


def _install_profile_hook():
    """The agent image's antenv lacks axon_hooks; shim it so
    run_bass_kernel_spmd(trace=True) can capture NTFF profiles."""
    import sys
    import types
    if "antenv.axon_hooks" in sys.modules:
        return True
    try:
        from trn_agent_boot.trn_boot import _ntff_profile_via_ctypes
        hook = _ntff_profile_via_ctypes("/opt/axon/libaxon_pjrt.so")
        if hook is None:
            return False
        mod = types.ModuleType("antenv.axon_hooks")
        mod._hook = hook
        mod.get_axon_ntff_profile_hook = lambda: mod._hook

        def _set(h):
            mod._hook = h
        mod.set_axon_ntff_profile_hook = _set
        sys.modules["antenv.axon_hooks"] = mod
        import antenv
        antenv.axon_hooks = mod
        return True
    except Exception as e:  # profiling is best-effort
        print(f"profile hook unavailable: {e}")
        return False


# ---------------------------------------------------------------- host prep
def _preprocess(x, W1, b1, W2, b2, edge_src, edge_dst, batch):
    import ml_dtypes
    f32 = np.float32
    bf16 = ml_dtypes.bfloat16
    src = np.asarray(edge_src, np.int64)
    dst = np.asarray(edge_dst, np.int64)
    bat = np.asarray(batch, np.int64)
    x = np.asarray(x, f32)

    deg = np.bincount(dst, minlength=N).astype(np.float64) + 1.0
    dis = 1.0 / np.sqrt(deg)
    cnt = np.maximum(np.bincount(bat, minlength=G), 1).astype(np.float64)

    # per-core degree-descending node permutation
    pos = np.empty(N, np.int64)
    order = np.empty(N, np.int64)      # order[k*NV+j] = node at position j
    for k in range(P):
        v0 = k * NV
        loc = np.argsort(-deg[v0:v0 + NV], kind="stable")
        order[v0:v0 + NV] = v0 + loc
        pos[v0 + loc] = np.arange(NV)

    # ---- layer-1 gather edges (no self-loops; those come from local h
    # tiles via an identity matmul), grouped (core, tile, src-table)
    d_own = dst // NV
    d_pos = pos[dst]
    t_of = d_pos // 128
    dloc_v = (d_pos % 128).astype(f32)
    s_core = src // NV
    s_pos = pos[src]
    is_B = (s_pos >= ROWS_A).astype(np.int64)
    idx_v = np.where(is_B, s_core * ROWS_B + (s_pos - ROWS_A),
                     s_core * ROWS_A + s_pos).astype(np.int16)

    key = (d_own * NT + t_of) * 2 + is_B
    ordr = np.argsort(key, kind="stable")
    idx_s = idx_v[ordr]
    dloc_s = dloc_v[ordr]
    bounds = np.searchsorted(key[ordr], np.arange(P * NT * 2 + 1))
    cnts = np.diff(bounds).reshape(P, NT, 2)
    CH = -(-cnts // 128)               # chunks per (core, tile, table)
    CH = CH.max(axis=0)                # [NT, 2]  uniform across cores

    # batches: stride-interleaved tiles so per-batch work is balanced
    tiles_of_batch = [[t for t in range(NT) if t % NB == b] for b in range(NB)]

    # chunk-column layout: per batch, chunks grouped PER TILE (A then B)
    # so each tile's selector build is one contiguous is_eq op.
    # gather-index layout: per batch, [A chunks tile-major][B chunks
    # tile-major] (matches the two dma_gather calls).
    batch_meta = []        # per batch: dict(nA, nB, icol_A, icol_B)
    tile_meta = {}         # per tile: dict(col0, nA, nB, jA, jB)
    col = 0
    icol = 0
    for b in range(NB):
        nA = int(sum(CH[t, 0] for t in tiles_of_batch[b]))
        nB = int(sum(CH[t, 1] for t in tiles_of_batch[b]))
        batch_meta.append(dict(nA=nA, nB=nB,
                               icol_A=icol, icol_B=icol + nA * 8))
        jA = 0
        jB = 0
        for t in tiles_of_batch[b]:
            tile_meta[t] = dict(col0=col, nA=int(CH[t, 0]), nB=int(CH[t, 1]),
                                jA=jA, jB=jB)
            col += int(CH[t, 0]) + int(CH[t, 1])
            jA += int(CH[t, 0])
            jB += int(CH[t, 1])
        icol += (nA + nB) * 8
    NCH = col
    NIDX = NCH * 128

    # per-core data arrays
    xT = np.zeros((P, 128, TPAD), bf16)
    disc = np.zeros((P, 128, NT), f32)
    qb = np.zeros((P, TPAD, GP), f32)
    dloc_all = np.full((P, 128, NCH), -1.0, bf16)
    idx_flat = np.zeros((P, NIDX), np.int16)

    for k in range(P):
        ok = order[k * NV:(k + 1) * NV]
        xT[k, :, :NV] = (x[ok] * dis[ok, None]).T.astype(bf16)
        d = np.zeros(TPAD, f32)
        d[:NV] = dis[ok].astype(f32)
        disc[k] = d.reshape(NT, 128).T

    # fill chunk idx / dloc tables (idx layout: per batch, A tile-major
    # then B tile-major; dloc layout: per batch, per tile A then B)
    for b in range(NB):
        m = batch_meta[b]
        for h, base_icol in ((0, m["icol_A"]), (1, m["icol_B"])):
            jh = 0
            for t in tiles_of_batch[b]:
                nchunk = int(CH[t, h])
                if nchunk > 0:
                    tm = tile_meta[t]
                    for k in range(P):
                        gi = (k * NT + t) * 2 + h
                        g0, g1 = bounds[gi], bounds[gi + 1]
                        n = g1 - g0
                        fbase = base_icol * 16 + jh * 128
                        idx_flat[k, fbase:fbase + n] = idx_s[g0:g1]
                        pp = np.arange(n) % 128
                        cc = np.arange(n) // 128
                        colbase = tm["col0"] + (0 if h == 0 else tm["nA"])
                        dloc_all[k, pp, colbase + cc] = \
                            dloc_s[g0:g1].astype(bf16)
                jh += nchunk
    assert idx_flat.min() >= 0
    assert int(idx_flat.reshape(-1).max()) < P * ROWS_A
    # wrap gather indices: i -> [i % 16, i // 16], replicated to 128 partitions
    idxs = np.tile(
        idx_flat.reshape(P, NIDX // 16, 16).transpose(0, 2, 1), (1, 8, 1)
    ).astype(np.int16)

    # ---- layer-2 Q blocks: qb[core, pos[src], g] += norm/cnt[g]
    # (self-loops included here)
    e_src = np.concatenate([src, np.arange(N)])
    e_dst = np.concatenate([dst, np.arange(N)])
    g_of = bat[e_dst]
    val = (dis[e_src] * dis[e_dst] / cnt[g_of]).astype(f32)
    np.add.at(qb, (e_src // NV, pos[e_src], g_of), val)
    qb = qb.astype(bf16)

    iota_bf = np.broadcast_to(
        np.arange(128, dtype=bf16), (128, 128)).copy()
    eye16 = np.eye(16, dtype=f32)
    eye128 = np.eye(128, dtype=bf16)
    widx = np.zeros((128, 8), np.int16)

    W1 = np.ascontiguousarray(np.asarray(W1, f32).astype(bf16))
    W2 = np.ascontiguousarray(np.asarray(W2, f32))
    b1 = np.asarray(b1, f32)
    b2 = np.asarray(b2, f32)
    use_b1 = bool(np.any(b1))
    use_b2 = bool(np.any(b2))

    in_maps = []
    for k in range(P):
        m = {
            "xT": np.ascontiguousarray(xT[k]),
            "qb": np.ascontiguousarray(qb[k]),
            "idxs": np.ascontiguousarray(idxs[k]),
            "dloc": np.ascontiguousarray(dloc_all[k]),
            "disc": np.ascontiguousarray(disc[k]),
            "eye128": eye128,
            "w1": W1, "w2": W2,
            "iota": iota_bf, "eye16": eye16, "widx": widx,
        }
        if use_b1:
            rr = np.zeros((1, TPAD), f32)
            rr[0, :NV] = np.sqrt(deg[order[k * NV:(k + 1) * NV]]).astype(f32)
            m["rdis"] = rr
            m["b1r"] = b1.reshape(1, F)
        if use_b2:
            m["b2r"] = b2.reshape(1, C)
        in_maps.append(m)

    plan = dict(NCH=NCH, NIDX=NIDX, CH=CH, tiles_of_batch=tiles_of_batch,
                batch_meta=batch_meta, tile_meta=tile_meta,
                use_b1=use_b1, use_b2=use_b2)
    return plan, in_maps


# ---------------------------------------------------------------- bass build
def _build(plan):
    dt = mybir.dt
    f32, bf16, i16 = dt.float32, dt.bfloat16, dt.int16
    NCH, NIDX = plan["NCH"], plan["NIDX"]
    use_b1, use_b2 = plan["use_b1"], plan["use_b2"]
    single_packet = bool(int(os.environ.get("GCN_SP", "0")))

    stage = int(os.environ.get("GCN_STAGE", "3"))  # 1: no phase C; 2: +gathers
    nc = bacc.Bacc("TRN2", target_bir_lowering=False, debug=False,
                   num_devices=P, num_swdge_queues=NQ)
    xT_d = nc.dram_tensor("xT", [128, TPAD], bf16, kind="ExternalInput")
    qb_d = nc.dram_tensor("qb", [TPAD, GP], bf16, kind="ExternalInput")
    idxs_d = nc.dram_tensor("idxs", [128, NIDX // 16], i16, kind="ExternalInput")
    dloc_d = nc.dram_tensor("dloc", [128, NCH], bf16, kind="ExternalInput")
    disc_d = nc.dram_tensor("disc", [128, NT], f32, kind="ExternalInput")
    eye128_d = nc.dram_tensor("eye128", [128, 128], bf16, kind="ExternalInput")
    w1_d = nc.dram_tensor("w1", [F, HID], bf16, kind="ExternalInput")
    w2_d = nc.dram_tensor("w2", [HID, C], f32, kind="ExternalInput")
    iota_d = nc.dram_tensor("iota", [128, 128], bf16, kind="ExternalInput")
    eye_d = nc.dram_tensor("eye16", [16, 16], f32, kind="ExternalInput")
    widx_d = nc.dram_tensor("widx", [128, 8], i16, kind="ExternalInput")
    if use_b1:
        rdis_d = nc.dram_tensor("rdis", [1, TPAD], f32, kind="ExternalInput")
        b1_d = nc.dram_tensor("b1r", [1, F], f32, kind="ExternalInput")
    if use_b2:
        b2_d = nc.dram_tensor("b2r", [1, C], f32, kind="ExternalInput")
    y_d = nc.dram_tensor("y", [G, C], f32, kind="ExternalOutput")

    with tile.TileContext(nc) as tc:
        cpool = tc.alloc_tile_pool(name="const", bufs=1)
        dram = tc.alloc_tile_pool(name="dram", bufs=1, space="DRAM")

        h_ownA = dram.tile([ROWS_A, F], bf16)
        h_ownB = dram.tile([ROWS_B, F], bf16)
        h_fullA = dram.tile([P * ROWS_A, F], bf16, addr_space="Shared")
        h_fullB = dram.tile([P * ROWS_B, F], bf16, addr_space="Shared")
        ar_in = dram.tile([128, GP], f32)
        ar_out = dram.tile([128, GP], f32)

        # warm gather: preloads the Q7 ext-isa library (~9us) before it's
        # needed; reads a fixed xT row, result unused.
        widx_sb = cpool.tile([128, 8], i16)
        nc.sync.dma_start(widx_sb[:], widx_d[:, :])
        warm_sb = cpool.tile([128, 1, 128], bf16)
        if int(os.environ.get("GCN_WARM", "1")):
            nc.gpsimd.dma_gather(
                out_ap=warm_sb[:], in_ap=xT_d[:, 0:128],
                idxs_ap=widx_sb[:, :], num_idxs=128, num_idxs_reg=128,
                elem_size=F, elem_step=TPAD, single_packet=False,
                queue_num=0)

        # phase-B-critical constants first so their DMAs run first
        w1_sb = cpool.tile([F, HID], bf16)
        nc.sync.dma_start(w1_sb[:], w1_d[:, :])
        disc_sb = cpool.tile([128, NT], f32)
        nc.sync.dma_start(disc_sb[:], disc_d[:, :])
        h_loc = cpool.tile([128, TPAD], bf16)   # local dis*h, node-major tiles
        h1_sb = cpool.tile([128, TPAD], bf16)
        idxs_sb = cpool.tile([128, NIDX // 16], i16)
        nc.scalar.dma_start(idxs_sb[:], idxs_d[:, :])
        dloc_sb = cpool.tile([128, NCH], bf16)
        nc.scalar.dma_start(dloc_sb[:], dloc_d[:, :])

        # ---------------- phase B: h = dis * (x @ W1), 2 AllGathers
        with (
            tc.tile_pool(name="xw", bufs=1) as xw,
            tc.tile_pool(name="hp", bufs=4, space="PSUM") as hp,
        ):
            xT_sb = xw.tile([128, TPAD], bf16)
            nc.sync.dma_start(xT_sb[:], xT_d[:, :])
            for t in range(NT):
                ps = hp.tile([128, 128], f32)
                nc.tensor.matmul(ps[:], lhsT=xT_sb[:, t * 128:(t + 1) * 128],
                                 rhs=w1_sb[:], start=True, stop=True)
                if t % 2 == 0:
                    nc.scalar.activation(h_loc[:, t * 128:(t + 1) * 128],
                                         ps[:], AF.Copy)
                else:
                    nc.vector.tensor_copy(h_loc[:, t * 128:(t + 1) * 128],
                                          ps[:])
                eng = nc.sync if t % 2 == 0 else nc.scalar
                if t < NTA:
                    eng.dma_start(h_ownA[t * 128:(t + 1) * 128, :],
                                  h_loc[:, t * 128:(t + 1) * 128])
                else:
                    tb = t - NTA
                    eng.dma_start(h_ownB[tb * 128:(tb + 1) * 128, :],
                                  h_loc[:, t * 128:(t + 1) * 128])
                if t == NTA - 1:
                    nc.gpsimd.collective_compute(
                        "AllGather", ALU.bypass,
                        replica_groups=[list(range(P))],
                        ins=[h_ownA[:].opt()], outs=[h_fullA[:].opt()])
            nc.gpsimd.collective_compute(
                "AllGather", ALU.bypass, replica_groups=[list(range(P))],
                ins=[h_ownB[:].opt()], outs=[h_fullB[:].opt()])

        # remaining phase-C constants
        iota_sb = cpool.tile([128, 128], bf16)
        nc.sync.dma_start(iota_sb[:], iota_d[:, :])
        eye_sb = cpool.tile([16, 16], f32)
        nc.sync.dma_start(eye_sb[:], eye_d[:, :])
        eye128_sb = cpool.tile([128, 128], bf16)
        nc.sync.dma_start(eye128_sb[:], eye128_d[:, :])
        w2_sb = cpool.tile([HID, C], f32)
        nc.sync.dma_start(w2_sb[:], w2_d[:, :])
        if use_b1:
            rdis_sb = cpool.tile([1, TPAD], f32)
            nc.sync.dma_start(rdis_sb[:], rdis_d[:, :])
            b1_sb = cpool.tile([1, F], f32)
            nc.sync.dma_start(b1_sb[:], b1_d[:, :])
        if use_b2:
            b2_sb = cpool.tile([1, C], f32)
            nc.sync.dma_start(b2_sb[:], b2_d[:, :])

        # ---------------- phase C: layer-1 aggregation + layer-2 contraction
        with tc.tile_pool(name="ptp", bufs=1, space="PSUM") as ptp:
            poolT = ptp.tile([128, GP], f32)
            i_l2 = 0
            with (
                tc.tile_pool(name="ga", bufs=5) as ga_p,
                tc.tile_pool(name="gb", bufs=5) as gb_p,
                tc.tile_pool(name="selp", bufs=6) as selp,
                tc.tile_pool(name="qp", bufs=3) as qp,
                tc.tile_pool(name="aggp", bufs=7, space="PSUM") as aggp,
            ):
                for b in range(NB):
                    m = plan["batch_meta"][b]
                    nA, nB = m["nA"], m["nB"]
                    ngb = int(os.environ.get("GCN_NGB", str(NB)))
                    gA = gB = None
                    if b >= ngb:
                        continue
                    if nA and stage >= 2:
                        gA = ga_p.tile([128, nA, 128], bf16, tag="ga")
                        nc.gpsimd.dma_gather(
                            out_ap=gA[:], in_ap=h_fullA[:, :],
                            idxs_ap=idxs_sb[:, m["icol_A"]:
                                            m["icol_A"] + nA * 8],
                            num_idxs=nA * 128, num_idxs_reg=nA * 128,
                            elem_size=F, single_packet=single_packet,
                            queue_num=b % NQ)
                    if nB and stage >= 2:
                        gB = gb_p.tile([128, nB, 128], bf16, tag="gb")
                        nc.gpsimd.dma_gather(
                            out_ap=gB[:], in_ap=h_fullB[:, :],
                            idxs_ap=idxs_sb[:, m["icol_B"]:
                                            m["icol_B"] + nB * 8],
                            num_idxs=nB * 128, num_idxs_reg=nB * 128,
                            elem_size=F, single_packet=single_packet,
                            queue_num=(b + 2) % NQ)
                    if stage < 3:
                        if stage == 2 and (gA is not None or gB is not None):
                            junk = selp.tile([128, 128], bf16, tag="sel")
                            gj = gA if gA is not None else gB
                            nc.vector.tensor_copy(junk[:], gj[:, 0, :])
                            nc.sync.dma_start(ar_in[0:128, 0:64],
                                              junk[:, 0:64])
                        continue
                    for t in plan["tiles_of_batch"][b]:
                        tm = plan["tile_meta"][t]
                        t_nA, t_nB = tm["nA"], tm["nB"]
                        nch_t = t_nA + t_nB
                        # one is_eq builds all selectors for this tile
                        sel = selp.tile([128, nch_t, 128], bf16, tag="sel")
                        nc.vector.tensor_tensor(
                            out=sel[:],
                            in0=iota_sb[:, None, :].to_broadcast(
                                [128, nch_t, 128]),
                            in1=dloc_sb[:, tm["col0"]:tm["col0"] + nch_t,
                                        None].to_broadcast([128, nch_t, 128]),
                            op=ALU.is_equal)
                        ps = aggp.tile([128, 128], f32, tag="agg")
                        # self-loop: the message is h_loc itself; identity
                        # lhsT adds it into the PSUM accumulation
                        nc.tensor.matmul(
                            ps[:], lhsT=eye128_sb[:],
                            rhs=h_loc[:, t * 128:(t + 1) * 128],
                            start=True, stop=False)
                        if use_b1:
                            nc.tensor.matmul(
                                ps[:], lhsT=rdis_sb[0:1, t * 128:(t + 1) * 128],
                                rhs=b1_sb[:], start=False, stop=False)
                        for ci in range(nch_t):
                            if ci < t_nA:
                                gsrc, joff = gA, tm["jA"] + ci
                            else:
                                gsrc, joff = gB, tm["jB"] + (ci - t_nA)
                            nc.tensor.matmul(
                                ps[:], lhsT=sel[:, ci, :],
                                rhs=gsrc[:, joff, :],
                                start=False, stop=(ci == nch_t - 1))
                        nc.scalar.activation(
                            h1_sb[:, t * 128:(t + 1) * 128], ps[:], AF.Relu,
                            scale=disc_sb[:, t:t + 1])
                        # layer 2: poolT += H1_tile^T-contraction with Q block
                        qt = qp.tile([128, GP], bf16, tag="q")
                        nc.sync.dma_start(
                            qt[:], qb_d[t * 128:(t + 1) * 128, :])
                        nc.tensor.matmul(
                            poolT[:],
                            lhsT=h1_sb[:, t * 128:(t + 1) * 128],
                            rhs=qt[:],
                            start=(i_l2 == 0), stop=(i_l2 == NT - 1))
                        i_l2 += 1

            pt_sb = cpool.tile([128, GP], f32)
            if stage >= 3:
                nc.scalar.activation(pt_sb[:], poolT[:], AF.Copy)
            else:
                nc.vector.memset(pt_sb[:], 0.0)
            nc.sync.dma_start(ar_in[:], pt_sb[:])

        nc.gpsimd.collective_compute(
            "AllReduce", ALU.add, replica_groups=[list(range(P))],
            ins=[ar_in[:].opt()], outs=[ar_out[:].opt()])

        # ---------------- phase D: W2, bias, log_softmax
        with (
            tc.tile_pool(name="fin", bufs=1) as fin,
            tc.tile_pool(name="fps", bufs=2, space="PSUM") as fps,
            tc.tile_pool(name="sm", bufs=4) as smp,
        ):
            pooledT = fin.tile([128, GP], f32)
            nc.sync.dma_start(pooledT[:], ar_out[:])
            out2 = fps.tile([16, GP], f32, tag="out2")
            nc.tensor.matmul(out2[:], lhsT=w2_sb[:], rhs=pooledT[:],
                             start=True, stop=not use_b2)
            if use_b2:
                ones = fin.tile([1, GP], f32)
                nc.vector.memset(ones[:], 1.0)
                nc.tensor.matmul(out2[:], lhsT=b2_sb[:], rhs=ones[:],
                                 start=False, stop=True)
            logitsT = fin.tile([16, GP], f32)
            nc.scalar.activation(logitsT[:], out2[:], AF.Copy)
            for gt in range(min(GT, -(-G // 128))):
                tp = fps.tile([128, 16], f32, tag="tp")
                nc.tensor.transpose(
                    tp[:], logitsT[:, gt * 128:(gt + 1) * 128], eye_sb[:])
                nmx = smp.tile([128, 1], f32, tag="nmx")
                nc.vector.reduce_max(out=nmx[:], in_=tp[:],
                                     axis=mybir.AxisListType.X, negate=True)
                ex = smp.tile([128, 16], f32, tag="ex")
                nc.scalar.activation(ex[:], tp[:], AF.Exp, bias=nmx[:, 0:1])
                sm = smp.tile([128, 1], f32, tag="sm")
                nc.vector.reduce_sum(out=sm[:], in_=ex[:],
                                     axis=mybir.AxisListType.X)
                lse = smp.tile([128, 1], f32, tag="lse")
                nc.scalar.activation(lse[:], sm[:], AF.Ln)
                res = smp.tile([128, 16], f32, tag="res")
                nc.vector.tensor_scalar(res[:], tp[:], nmx[:, 0:1],
                                        lse[:, 0:1], ALU.add, ALU.subtract)
                rows = min(128, G - gt * 128)
                nc.sync.dma_start(y_d[gt * 128:gt * 128 + rows, :],
                                  res[0:rows, :])
        dram.release()
        cpool.release()
    nc.compile()
    return nc


# ---------------------------------------------------------------- entry
def kernel(x, W1, b1, W2, b2, edge_src, edge_dst, batch):
    global LAST_EXEC_NS, LAST_RESULT
    plan, in_maps = _preprocess(x, W1, b1, W2, b2,
                                edge_src, edge_dst, batch)
    nc = _build(plan)
    trace = bool(int(os.environ.get("GCN_TRACE", "0")))
    kw = {}
    if trace and _install_profile_hook():
        kw = dict(trace=True, trace_cores=[0])
    res = run_bass_kernel_spmd(nc, in_maps, core_ids=list(range(P)), **kw)
    LAST_RESULT = res
    LAST_EXEC_NS = res.exec_time_ns
    return np.ascontiguousarray(res.results[0]["y"].astype(np.float32))
